# revision 3
# baseline (speedup 1.0000x reference)
"""Trainium2 Bass kernel for nn_Attention_1580547974448.

Math insight: the reference uses raw .reshape (not a head-split transpose) on
[B,T,H*HD] -> [B,H,T,HD].  With B=4, T=4096, DIM=1024, H=16, HD=64 this makes
each "head" a contiguous 256-row slab of the flattened [B*T, DIM] = [16384,1024]
input: for slab s (rows 256s..256s+255),
    Q = (x_s @ Wq + bq)            viewed row-major as [4096, 64]
    S = Q^T K / sqrt(64)           [64, 64]
    P = softmax(S, axis=-1)
    O = P @ V^T                    [64, 4096], row-major == [256, 1024]
    y_s = O_v @ Wp + bp
i.e. the whole computation is block-diagonal over 64 independent slabs.
We shard 8 slabs (2048 rows) per NeuronCore -> pure data parallel, no
collectives.  Compute dtype bf16 (fp32 PSUM accumulation).

Per-slab device dataflow (all layouts [partition, free]):
  xt       [128, 8kd, 2048]   x^T, bf16 (host pre-transposed)
  q_nat    [128, 2rt, 1024]   Q rows-on-partitions   = xt_slab^T @ Wq + bq
  k_nat    same for K
  S psum   [64, 64]           sum over (rt,t2) of Q[:,64t2:+64]^T @ K[:,64t2:+64]
  softmax  on free dim; W bf16; WT = W^T via PE transpose
  vt       [128, 8jt, 256]    V^T (features on partitions) = Wv^T @ xt_slab
  vvt      [64, 256t1, 16t2]  Vv^T; flat free index = 16*t1+t2 = t
  O^T      chunks [128, 64] = vvt[:, 128c:+128]^T @ WT, evac'd strided into
  ovt      [128, 8ct, 256]    Ov^T (cols r = 4d + t3)
  y        [128, 2rt, 1024]   = ovt^T @ Wp + bp -> DMA to out rows
"""

import os
import sys

import numpy as np
import ml_dtypes

import concourse.bass as bass
import concourse.mybir as mybir
import concourse.tile as tile
from concourse import bacc
from concourse.bass_utils import run_bass_kernel_spmd


def _install_ntff_hook_shim():
    """concourse's trace path does `from antenv.axon_hooks import
    get_axon_ntff_profile_hook`; this container's antenv lacks that
    module.  Provide it: a ctypes hook on the axon PJRT .so when
    available (mirrors trn_agent_boot), else a None hook (concourse
    then skips tracing gracefully)."""
    try:
        import antenv.axon_hooks  # noqa: F401
        return
    except ImportError:
        pass
    import contextlib
    import ctypes
    import types

    state = {"hook": None}

    def _build_hook():
        so_path = "/opt/axon/libaxon_pjrt.so"
        if not os.path.exists(so_path):
            return None
        lib = ctypes.CDLL(so_path)
        if not hasattr(lib, "axon_start_nrt_profile"):
            return None
        lib.axon_start_nrt_profile.argtypes = [
            ctypes.POINTER(ctypes.c_int64), ctypes.c_size_t]
        lib.axon_start_nrt_profile.restype = ctypes.c_int64
        lib.axon_stop_nrt_profile.argtypes = [ctypes.c_char_p]
        lib.axon_stop_nrt_profile.restype = ctypes.c_int64

        @contextlib.contextmanager
        def _hook(output_dir, device_ids):
            import jax
            jax.devices()
            if device_ids:
                ids = (ctypes.c_int64 * len(device_ids))(*device_ids)
                rc = lib.axon_start_nrt_profile(ids, len(device_ids))
            else:
                rc = lib.axon_start_nrt_profile(None, 0)
            if rc != 0:
                raise RuntimeError(f"axon_start_nrt_profile rc={rc}")
            try:
                yield
            finally:
                n = lib.axon_stop_nrt_profile(str(output_dir).encode())
                if n < 0:
                    raise RuntimeError(f"axon_stop_nrt_profile rc={n}")
                print(f"profile: {n} file(s) written to {output_dir}")

        return _hook

    def get_axon_ntff_profile_hook():
        if state["hook"] is None:
            try:
                state["hook"] = _build_hook()
            except Exception:
                state["hook"] = None
        return state["hook"]

    mod = types.ModuleType("antenv.axon_hooks")
    mod.get_axon_ntff_profile_hook = get_axon_ntff_profile_hook
    mod.set_axon_ntff_profile_hook = lambda h: state.update(hook=h)
    sys.modules["antenv.axon_hooks"] = mod


_install_ntff_hook_shim()

P = 128          # SBUF partitions
DIM = 1024       # model dim
KD = DIM // P    # 8 contraction tiles
ROWS_PER_CORE = 2048
SLABS_PER_CORE = 8
SLAB = 256       # rows per slab
N_CORES = 8
BF16 = mybir.dt.bfloat16
F32 = mybir.dt.float32

_CACHE = {}


def _build_graph():
    nc = bacc.Bacc("TRN2", target_bir_lowering=False, debug=False,
                   num_devices=N_CORES)

    xt_d = nc.dram_tensor("xt", [DIM, ROWS_PER_CORE], BF16, kind="ExternalInput")
    w_d = {
        name: nc.dram_tensor(name, [DIM, DIM], BF16, kind="ExternalInput")
        for name in ("wq", "wk", "wv", "wp")
    }
    bqc_d = nc.dram_tensor("bqc", [P, DIM], F32, kind="ExternalInput")
    bkc_d = nc.dram_tensor("bkc", [P, DIM], F32, kind="ExternalInput")
    bpc_d = nc.dram_tensor("bpc", [P, DIM], F32, kind="ExternalInput")
    bvc_d = nc.dram_tensor("bvc", [P, KD], F32, kind="ExternalInput")
    ident_d = nc.dram_tensor("ident64", [64, 64], BF16, kind="ExternalInput")
    out_d = nc.dram_tensor("out", [ROWS_PER_CORE, DIM], F32, kind="ExternalOutput")

    with tile.TileContext(nc) as tc:
        with (
            tc.tile_pool(name="wpool", bufs=1) as wpool,
            tc.tile_pool(name="xpool", bufs=1) as xpool,
            tc.tile_pool(name="bias", bufs=1) as bias_pool,
            tc.tile_pool(name="qk", bufs=2) as qk_pool,
            tc.tile_pool(name="vt", bufs=2) as vt_pool,
            tc.tile_pool(name="vvt", bufs=2) as vvt_pool,
            tc.tile_pool(name="ovt", bufs=2) as ovt_pool,
            tc.tile_pool(name="ysb", bufs=2) as y_pool,
            tc.tile_pool(name="soft", bufs=2) as soft_pool,
            tc.tile_pool(name="ps_proj", bufs=3, space="PSUM") as ps_proj_pool,
            tc.tile_pool(name="ps_s", bufs=2, space="PSUM") as ps_s_pool,
            tc.tile_pool(name="ps_ot", bufs=2, space="PSUM") as ps_ot_pool,
            tc.tile_pool(name="ps_wt", bufs=1, space="PSUM") as ps_wt_pool,
        ):
            # ---- resident tensors -------------------------------------------
            w_sb = {}
            for name in ("wq", "wk", "wv", "wp"):
                t = wpool.tile([P, KD, DIM], BF16, tag=f"w_{name}")
                src = w_d[name][:].rearrange("(kd p) c -> p kd c", p=P)
                for kd in range(KD):
                    nc.sync.dma_start(t[:, kd, :], src[:, kd, :])
                w_sb[name] = t

            xt_sb = xpool.tile([P, KD, ROWS_PER_CORE], BF16, tag="xt")
            xt_src = xt_d[:].rearrange("(kd p) r -> p kd r", p=P)
            for kd in range(KD):
                nc.sync.dma_start(xt_sb[:, kd, :], xt_src[:, kd, :])

            bq_bc = bias_pool.tile([P, DIM], F32, tag="bqc")
            bk_bc = bias_pool.tile([P, DIM], F32, tag="bkc")
            bp_bc = bias_pool.tile([P, DIM], F32, tag="bpc")
            bv_col = bias_pool.tile([P, KD], F32, tag="bvc")
            ident = bias_pool.tile([64, 64], BF16, tag="ident")
            nc.sync.dma_start(bq_bc[:], bqc_d[:])
            nc.sync.dma_start(bk_bc[:], bkc_d[:])
            nc.sync.dma_start(bp_bc[:], bpc_d[:])
            nc.sync.dma_start(bv_col[:], bvc_d[:])
            nc.sync.dma_start(ident[:], ident_d[:])

            # ---- per-slab pipeline ------------------------------------------
            for s in range(SLABS_PER_CORE):
                c0 = s * SLAB  # xt column base of this slab

                # Q, K natural layout (rows on partitions)
                q_nat = qk_pool.tile([P, 2, DIM], BF16, tag="q_nat")
                k_nat = qk_pool.tile([P, 2, DIM], BF16, tag="k_nat")
                for dst_t, wname, bias_bc in (
                    (q_nat, "wq", bq_bc),
                    (k_nat, "wk", bk_bc),
                ):
                    for rt in range(2):
                        for jc in range(2):
                            ps = ps_proj_pool.tile([P, 512], F32, tag="ps_proj")
                            for kd in range(KD):
                                nc.tensor.matmul(
                                    ps[:],
                                    xt_sb[:, kd, c0 + rt * P: c0 + (rt + 1) * P],
                                    w_sb[wname][:, kd, jc * 512:(jc + 1) * 512],
                                    start=(kd == 0),
                                    stop=(kd == KD - 1),
                                )
                            nc.vector.tensor_add(
                                dst_t[:, rt, jc * 512:(jc + 1) * 512],
                                ps[:],
                                bias_bc[:, jc * 512:(jc + 1) * 512],
                            )

                # S = sum over (rt, t2) of Q_blk^T @ K_blk  -> PSUM [64, 64]
                ps_s = ps_s_pool.tile([64, 64], F32, tag="ps_s")
                n_acc = 0
                for rt in range(2):
                    for t2 in range(16):
                        nc.tensor.matmul(
                            ps_s[:],
                            q_nat[:, rt, t2 * 64:(t2 + 1) * 64],
                            k_nat[:, rt, t2 * 64:(t2 + 1) * 64],
                            start=(n_acc == 0),
                            stop=(n_acc == 31),
                        )
                        n_acc += 1

                # softmax over the free dim (runs on DVE/ACT while PE does VT)
                negmax = soft_pool.tile([64, 1], F32, tag="negmax")
                nc.vector.reduce_max(negmax[:], ps_s[:],
                                     axis=mybir.AxisListType.X, negate=True)
                p_sb = soft_pool.tile([64, 64], F32, tag="p_sb")
                rsum = soft_pool.tile([64, 1], F32, tag="rsum")
                nc.scalar.activation(p_sb[:], ps_s[:],
                                     mybir.ActivationFunctionType.Exp,
                                     bias=negmax[:], accum_out=rsum[:])
                rinv = soft_pool.tile([64, 1], F32, tag="rinv")
                nc.vector.reciprocal(rinv[:], rsum[:])
                w_soft = soft_pool.tile([64, 64], BF16, tag="w_soft")
                nc.vector.tensor_scalar_mul(w_soft[:], p_sb[:], rinv[:])

                # V^T projection (features on partitions)
                vt = vt_pool.tile([P, KD, SLAB], BF16, tag="vt")
                for jt in range(KD):
                    ps = ps_proj_pool.tile([P, SLAB], F32, tag="ps_proj")
                    for kd in range(KD):
                        nc.tensor.matmul(
                            ps[:],
                            w_sb["wv"][:, kd, jt * P:(jt + 1) * P],
                            xt_sb[:, kd, c0: c0 + SLAB],
                            start=(kd == 0),
                            stop=(kd == KD - 1),
                        )
                    nc.vector.tensor_scalar_add(
                        vt[:, jt, :], ps[:], bv_col[:, jt: jt + 1])

                # VvT[e, t1, t2] = VT[64*t2+e, t1]   (flat free = 16*t1+t2 = t)
                vvt = vvt_pool.tile([64, SLAB, 16], BF16, tag="vvt")
                for t2 in range(16):
                    nc.vector.tensor_copy(
                        vvt[:, :, t2],
                        vt[(t2 % 2) * 64:(t2 % 2) * 64 + 64, t2 // 2, :],
                    )

                # WT = W^T via PE transpose (after VT so PE doesn't stall on
                # the softmax chain)
                ps_wt = ps_wt_pool.tile([64, 64], BF16, tag="ps_wt")
                nc.tensor.transpose(ps_wt[:], w_soft[:], ident[:])
                wt_sb = soft_pool.tile([64, 64], BF16, tag="wt_sb")
                nc.vector.tensor_copy(wt_sb[:], ps_wt[:])

                # O^T chunks [128t, 64d], evac fused into OvT interleave:
                # chunk c covers t in [128c, 128c+128): OvT col r = 4d + c//8,
                # OvT tile ct = c % 8.
                ovt = ovt_pool.tile([P, KD, SLAB], BF16, tag="ovt")
                vvt_flat = vvt[:].rearrange("e t1 t2 -> e (t1 t2)")
                for c in range(32):
                    pso = ps_ot_pool.tile([P, 64], F32, tag="ps_ot")
                    nc.tensor.matmul(
                        pso[:],
                        vvt_flat[:, c * P:(c + 1) * P],
                        wt_sb[:],
                        start=True, stop=True,
                    )
                    dst = ovt[:, c % 8, :] \
                        .rearrange("p (d four) -> p d four", four=4)[:, :, c // 8]
                    nc.vector.tensor_copy(dst, pso[:])

                # Y = OvT^T @ Wp + bp (natural rows) -> DMA out
                y_sb = y_pool.tile([P, 2, DIM], F32, tag="y_sb")
                for rt in range(2):
                    for jc in range(2):
                        ps = ps_proj_pool.tile([P, 512], F32, tag="ps_proj")
                        for ct in range(KD):
                            nc.tensor.matmul(
                                ps[:],
                                ovt[:, ct, rt * P:(rt + 1) * P],
                                w_sb["wp"][:, ct, jc * 512:(jc + 1) * 512],
                                start=(ct == 0),
                                stop=(ct == KD - 1),
                            )
                        nc.vector.tensor_add(
                            y_sb[:, rt, jc * 512:(jc + 1) * 512],
                            ps[:],
                            bp_bc[:, jc * 512:(jc + 1) * 512],
                        )

                out_dst = out_d[s * SLAB:(s + 1) * SLAB, :] \
                    .rearrange("(rt p) c -> p rt c", p=P)
                nc.sync.dma_start(out_dst, y_sb[:])

    nc.compile()
    return nc


def _prep_inputs(x, Wq, bq, Wk, bk, Wv, bv, Wp, bp):
    """Host-side shard prep. Returns in_maps list for 8 cores."""
    bf16 = ml_dtypes.bfloat16
    xf = np.ascontiguousarray(np.asarray(x, dtype=np.float32).reshape(-1, DIM))
    scale = np.float32(1.0 / np.sqrt(64.0))

    wq_b = np.ascontiguousarray((np.asarray(Wq) * scale).astype(bf16))
    wk_b = np.ascontiguousarray(np.asarray(Wk).astype(bf16))
    wv_b = np.ascontiguousarray(np.asarray(Wv).astype(bf16))
    wp_b = np.ascontiguousarray(np.asarray(Wp).astype(bf16))

    bqc = np.ascontiguousarray(np.broadcast_to(
        (np.asarray(bq) * scale).astype(np.float32), (P, DIM)))
    bkc = np.ascontiguousarray(np.broadcast_to(
        np.asarray(bk, dtype=np.float32), (P, DIM)))
    bpc = np.ascontiguousarray(np.broadcast_to(
        np.asarray(bp, dtype=np.float32), (P, DIM)))
    bvc = np.ascontiguousarray(
        np.asarray(bv, dtype=np.float32).reshape(KD, P).T)
    ident = np.eye(64, dtype=bf16)

    shared = {
        "wq": wq_b, "wk": wk_b, "wv": wv_b, "wp": wp_b,
        "bqc": bqc, "bkc": bkc, "bpc": bpc, "bvc": bvc,
        "ident64": ident,
    }
    in_maps = []
    for c in range(N_CORES):
        xs = xf[c * ROWS_PER_CORE:(c + 1) * ROWS_PER_CORE]  # [2048, 1024]
        xt = np.ascontiguousarray(xs.T.astype(bf16))        # [1024, 2048]
        in_maps.append({"xt": xt, **shared})
    return in_maps


def kernel(x, Wq, bq, Wk, bk, Wv, bv, Wp, bp):
    if "nc" not in _CACHE:
        _CACHE["nc"] = _build_graph()
    nc = _CACHE["nc"]

    in_maps = _prep_inputs(x, Wq, bq, Wk, bk, Wv, bv, Wp, bp)
    trace = bool(int(os.environ.get("ATHENA_TRACE", "0")))
    res = run_bass_kernel_spmd(nc, in_maps, core_ids=list(range(N_CORES)),
                               trace=trace)
    _CACHE["last_exec_time_ns"] = res.exec_time_ns

    out = np.concatenate([res.results[c]["out"] for c in range(N_CORES)], axis=0)
    return np.ascontiguousarray(out.reshape(np.asarray(x).shape)
                                .astype(np.float32))


# revision 11
# speedup vs baseline: 1.3256x; 1.3256x over previous
"""Trainium2 Bass kernel for nn_Attention_1580547974448.

Math insight: the reference uses raw .reshape (not a head-split transpose) on
[B,T,H*HD] -> [B,H,T,HD].  With B=4, T=4096, DIM=1024, H=16, HD=64 this makes
each "head" a contiguous 256-row slab of the flattened [B*T, DIM] = [16384,1024]
input: for slab s (rows 256s..256s+255),
    Q = (x_s @ Wq + bq)            viewed row-major as [4096, 64]
    S = Q^T K / sqrt(64)           [64, 64]
    P = softmax(S, axis=-1)
    O = P @ V^T                    [64, 4096], row-major == [256, 1024]
    y_s = O_v @ Wp + bp
i.e. the whole computation is block-diagonal over 64 independent slabs.
We shard 8 slabs (2048 rows) per NeuronCore -> pure data parallel, no
collectives.  Compute dtype bf16 (fp32 PSUM accumulation).

Per-core dataflow (all layouts [partition, free]):
  xt       [128, 8kd, 2048]   x^T, bf16 (host pre-transposed)
  per slab-pair (VT batched at N=512 so LDWEIGHTS hides under the stream):
    vt     [128, 8jt, 512]    V^T for 2 slabs = Wv^T @ xt_pair
  per slab:
    q_nat  [128, 2rt, 1024]   Q rows-on-partitions = xt_slab^T @ Wq + bq (DVE)
    k_nat  same for K
    S psum [64, 64]           sum over (rt,t2) of Q[:,64t2:+64]^T K[:,64t2:+64]
    softmax on free dim (DVE/ACT); WT = W^T via PE transpose
    vvt    [64, 16t2, 256t1]  Vv^T, filled by 16 SBUF->SBUF DMAs (partition
                              crossing move; keeps DVE free)
    O^T    4 chunks per ct into one PSUM [128, 4t3, 64d], single CAST evac
    ovt    [128, 8ct, 256]    Ov^T (col r = 4d + t3)
    y      [128, 2rt, 1024]   = ovt^T @ Wp + bp -> DMA out
Engine split: PE matmuls; DVE Q/K/Y bias-evacs + OT evacs + softmax; ACT VT
bias-evacs + exp; DMA vvt transform.
"""

import os
import sys

import numpy as np
import ml_dtypes

import concourse.bass as bass
import concourse.mybir as mybir
import concourse.tile as tile
from concourse import bacc
from concourse.bass_utils import run_bass_kernel_spmd


def _install_ntff_hook_shim():
    """concourse's trace path does `from antenv.axon_hooks import
    get_axon_ntff_profile_hook`; this container's antenv lacks that
    module.  Provide it: a ctypes hook on the axon PJRT .so when
    available (mirrors trn_agent_boot), else a None hook (concourse
    then skips tracing gracefully)."""
    try:
        import antenv.axon_hooks  # noqa: F401
        return
    except ImportError:
        pass
    import contextlib
    import ctypes
    import types

    state = {"hook": None}

    def _build_hook():
        so_path = "/opt/axon/libaxon_pjrt.so"
        if not os.path.exists(so_path):
            return None
        lib = ctypes.CDLL(so_path)
        if not hasattr(lib, "axon_start_nrt_profile"):
            return None
        lib.axon_start_nrt_profile.argtypes = [
            ctypes.POINTER(ctypes.c_int64), ctypes.c_size_t]
        lib.axon_start_nrt_profile.restype = ctypes.c_int64
        lib.axon_stop_nrt_profile.argtypes = [ctypes.c_char_p]
        lib.axon_stop_nrt_profile.restype = ctypes.c_int64

        @contextlib.contextmanager
        def _hook(output_dir, device_ids):
            import jax
            jax.devices()
            if device_ids:
                ids = (ctypes.c_int64 * len(device_ids))(*device_ids)
                rc = lib.axon_start_nrt_profile(ids, len(device_ids))
            else:
                rc = lib.axon_start_nrt_profile(None, 0)
            if rc != 0:
                raise RuntimeError(f"axon_start_nrt_profile rc={rc}")
            try:
                yield
            finally:
                n = lib.axon_stop_nrt_profile(str(output_dir).encode())
                if n < 0:
                    raise RuntimeError(f"axon_stop_nrt_profile rc={n}")
                print(f"profile: {n} file(s) written to {output_dir}")

        return _hook

    def get_axon_ntff_profile_hook():
        if state["hook"] is None:
            try:
                state["hook"] = _build_hook()
            except Exception:
                state["hook"] = None
        return state["hook"]

    mod = types.ModuleType("antenv.axon_hooks")
    mod.get_axon_ntff_profile_hook = get_axon_ntff_profile_hook
    mod.set_axon_ntff_profile_hook = lambda h: state.update(hook=h)
    sys.modules["antenv.axon_hooks"] = mod


_install_ntff_hook_shim()

P = 128          # SBUF partitions
DIM = 1024       # model dim
KD = DIM // P    # 8 contraction tiles
ROWS_PER_CORE = 2048
SLABS_PER_CORE = 8
SLAB = 256       # rows per slab
N_CORES = 8
BF16 = mybir.dt.bfloat16
F32 = mybir.dt.float32

_CACHE = {}


def _build_graph():
    nc = bacc.Bacc("TRN2", target_bir_lowering=False, debug=False,
                   num_devices=N_CORES)

    xt_d = nc.dram_tensor("xt", [DIM, ROWS_PER_CORE], BF16, kind="ExternalInput")
    w_d = {
        name: nc.dram_tensor(name, [DIM, DIM], BF16, kind="ExternalInput")
        for name in ("wq", "wk", "wv", "wp")
    }
    bqc_d = nc.dram_tensor("bqc", [P, DIM], F32, kind="ExternalInput")
    bkc_d = nc.dram_tensor("bkc", [P, DIM], F32, kind="ExternalInput")
    bpc_d = nc.dram_tensor("bpc", [P, DIM], F32, kind="ExternalInput")
    bvc_d = nc.dram_tensor("bvc", [P, KD], F32, kind="ExternalInput")
    ident_d = nc.dram_tensor("ident64", [64, 64], BF16, kind="ExternalInput")
    out_d = nc.dram_tensor("out", [ROWS_PER_CORE, DIM], F32, kind="ExternalOutput")

    with tile.TileContext(nc) as tc:
        with (
            tc.tile_pool(name="wpool", bufs=1) as wpool,
            tc.tile_pool(name="xpool", bufs=1) as xpool,
            tc.tile_pool(name="bias", bufs=1) as bias_pool,
            tc.tile_pool(name="qk", bufs=2) as qk_pool,
            tc.tile_pool(name="vt", bufs=2) as vt_pool,
            tc.tile_pool(name="vvt", bufs=2) as vvt_pool,
            tc.tile_pool(name="ovt", bufs=2) as ovt_pool,
            tc.tile_pool(name="ysb", bufs=2) as y_pool,
            tc.tile_pool(name="soft", bufs=2) as soft_pool,
            tc.tile_pool(name="ps_proj", bufs=4, space="PSUM") as ps_proj_pool,
            tc.tile_pool(name="ps_s", bufs=1, space="PSUM") as ps_s_pool,
            tc.tile_pool(name="ps_ot", bufs=2, space="PSUM") as ps_ot_pool,
            tc.tile_pool(name="ps_wt", bufs=1, space="PSUM") as ps_wt_pool,
        ):
            # ---- resident tensors -------------------------------------------
            bq_bc = bias_pool.tile([P, DIM], F32, tag="bqc")
            bk_bc = bias_pool.tile([P, DIM], F32, tag="bkc")
            bp_bc = bias_pool.tile([P, DIM], F32, tag="bpc")
            bv_col = bias_pool.tile([P, KD], F32, tag="bvc")
            ident = bias_pool.tile([64, 64], BF16, tag="ident")
            nc.sync.dma_start(bq_bc[:], bqc_d[:])
            nc.sync.dma_start(bk_bc[:], bkc_d[:])
            nc.sync.dma_start(bp_bc[:], bpc_d[:])
            nc.sync.dma_start(bv_col[:], bvc_d[:])
            nc.sync.dma_start(ident[:], ident_d[:])

            # fine-grained loads so slab-0 matmuls can start early
            xt_sb = xpool.tile([P, KD, ROWS_PER_CORE], BF16, tag="xt")
            xt_src = xt_d[:].rearrange("(kd p) r -> p kd r", p=P)
            w_sb = {}
            for name in ("wq", "wk", "wv", "wp"):
                w_sb[name] = wpool.tile([P, KD, DIM], BF16, tag=f"w_{name}",
                                        name=f"w_{name}")
            w_srcs = {name: w_d[name][:].rearrange("(kd p) c -> p kd c", p=P)
                      for name in w_sb}
            for kd in range(KD):
                nc.sync.dma_start(xt_sb[:, kd, 0:512], xt_src[:, kd, 0:512])
                nc.sync.dma_start(w_sb["wv"][:, kd, :], w_srcs["wv"][:, kd, :])
            for kd in range(KD):
                nc.sync.dma_start(w_sb["wq"][:, kd, :], w_srcs["wq"][:, kd, :])
                nc.sync.dma_start(w_sb["wk"][:, kd, :], w_srcs["wk"][:, kd, :])
            for kd in range(KD):
                nc.sync.dma_start(w_sb["wp"][:, kd, :], w_srcs["wp"][:, kd, :])
                for half in range(1, 4):
                    nc.sync.dma_start(
                        xt_sb[:, kd, half * 512:(half + 1) * 512],
                        xt_src[:, kd, half * 512:(half + 1) * 512])

            # ---- per slab-pair pipeline -------------------------------------
            for pair in range(SLABS_PER_CORE // 2):
                p0 = pair * 2 * SLAB  # xt column base of the pair

                # V^T for both slabs of the pair (N=512 keeps PE at stream rate)
                vt = vt_pool.tile([P, KD, 2 * SLAB], BF16, tag="vt")
                for jt in range(KD):
                    ps = ps_proj_pool.tile([P, 512], F32, tag="ps_proj")
                    for kd in range(KD):
                        nc.tensor.matmul(
                            ps[:],
                            w_sb["wv"][:, kd, jt * P:(jt + 1) * P],
                            xt_sb[:, kd, p0: p0 + 512],
                            start=(kd == 0),
                            stop=(kd == KD - 1),
                        )
                    # bias + cast on ScalarE (per-partition bias)
                    nc.scalar.activation(
                        vt[:, jt, :], ps[:],
                        mybir.ActivationFunctionType.Identity,
                        bias=bv_col[:, jt: jt + 1])

                for half in range(2):
                    s = pair * 2 + half
                    c0 = s * SLAB

                    # VvT[e, t1, t2] = VT[64*t2+e, t1] (flat free = 16*t1+t2 = t
                    # so matmul lhsT slices collapse to one free dim).  16
                    # partition-base-shift copies, split across ACT and DVE.
                    vvt = vvt_pool.tile([64, SLAB, 16], BF16, tag="vvt")
                    for t2 in range(16):
                        src = vt[(t2 % 2) * 64:(t2 % 2) * 64 + 64, t2 // 2,
                                 half * SLAB:(half + 1) * SLAB]
                        if t2 % 2 == 0:
                            nc.scalar.copy(vvt[:, :, t2], src)
                        else:
                            nc.vector.tensor_copy(vvt[:, :, t2], src)

                    # Q, K natural layout (rows on partitions)
                    q_nat = qk_pool.tile([P, 2, DIM], BF16, tag="q_nat")
                    k_nat = qk_pool.tile([P, 2, DIM], BF16, tag="k_nat")
                    for dst_t, wname, bias_bc in (
                        (q_nat, "wq", bq_bc),
                        (k_nat, "wk", bk_bc),
                    ):
                        for rt in range(2):
                            for jc in range(2):
                                ps = ps_proj_pool.tile([P, 512], F32,
                                                       tag="ps_proj")
                                for kd in range(KD):
                                    nc.tensor.matmul(
                                        ps[:],
                                        xt_sb[:, kd,
                                              c0 + rt * P: c0 + (rt + 1) * P],
                                        w_sb[wname][:, kd,
                                                    jc * 512:(jc + 1) * 512],
                                        start=(kd == 0),
                                        stop=(kd == KD - 1),
                                    )
                                nc.vector.tensor_add(
                                    dst_t[:, rt, jc * 512:(jc + 1) * 512],
                                    ps[:],
                                    bias_bc[:, jc * 512:(jc + 1) * 512],
                                )

                    # S = sum over (rt, t2) of Q_blk^T @ K_blk -> PSUM [64, 64]
                    ps_s = ps_s_pool.tile([64, 64], F32, tag="ps_s")
                    n_acc = 0
                    for rt in range(2):
                        for t2 in range(16):
                            nc.tensor.matmul(
                                ps_s[:],
                                q_nat[:, rt, t2 * 64:(t2 + 1) * 64],
                                k_nat[:, rt, t2 * 64:(t2 + 1) * 64],
                                start=(n_acc == 0),
                                stop=(n_acc == 31),
                            )
                            n_acc += 1

                    # softmax over the free dim (DVE/ACT, overlaps PE)
                    negmax = soft_pool.tile([64, 1], F32, tag="negmax")
                    nc.vector.reduce_max(negmax[:], ps_s[:],
                                         axis=mybir.AxisListType.X, negate=True)
                    p_sb = soft_pool.tile([64, 64], F32, tag="p_sb")
                    rsum = soft_pool.tile([64, 1], F32, tag="rsum")
                    nc.scalar.activation(p_sb[:], ps_s[:],
                                         mybir.ActivationFunctionType.Exp,
                                         bias=negmax[:], accum_out=rsum[:])
                    rinv = soft_pool.tile([64, 1], F32, tag="rinv")
                    nc.vector.reciprocal(rinv[:], rsum[:])
                    w_soft = soft_pool.tile([64, 64], BF16, tag="w_soft")
                    nc.vector.tensor_scalar_mul(w_soft[:], p_sb[:], rinv[:])

                    # WT = W^T via PE transpose
                    ps_wt = ps_wt_pool.tile([64, 64], BF16, tag="ps_wt")
                    nc.tensor.transpose(ps_wt[:], w_soft[:], ident[:])
                    wt_sb = soft_pool.tile([64, 64], BF16, tag="wt_sb")
                    nc.vector.tensor_copy(wt_sb[:], ps_wt[:])

                    # O^T chunks; 4 chunks (t3=0..3) of one ct share a PSUM
                    # tile [128, 4, 64], single CAST evac interleaves into ovt.
                    ovt = ovt_pool.tile([P, KD, SLAB], BF16, tag="ovt")
                    for ct in range(KD):
                        pso = ps_ot_pool.tile([P, 4, 64], F32, tag="ps_ot")
                        for t3 in range(4):
                            c = 8 * t3 + ct
                            # chunk c: t in [128c, 128c+128) -> t1 in [8c,8c+8)
                            # contiguous [64, 8, 16] -> opts to [64, 128]
                            lhs = vvt[:, c * 8:(c + 1) * 8, :]
                            nc.tensor.matmul(
                                pso[:, t3, :],
                                lhs,
                                wt_sb[:],
                                start=True, stop=True,
                            )
                        nc.vector.tensor_copy(
                            ovt[:, ct, :].rearrange("p (d four) -> p d four",
                                                    four=4),
                            pso[:].rearrange("p t3 d -> p d t3"),
                        )

                    # Y = OvT^T @ Wp + bp (natural rows) -> DMA out
                    y_sb = y_pool.tile([P, 2, DIM], F32, tag="y_sb")
                    for rt in range(2):
                        for jc in range(2):
                            ps = ps_proj_pool.tile([P, 512], F32, tag="ps_proj")
                            for ct in range(KD):
                                nc.tensor.matmul(
                                    ps[:],
                                    ovt[:, ct, rt * P:(rt + 1) * P],
                                    w_sb["wp"][:, ct, jc * 512:(jc + 1) * 512],
                                    start=(ct == 0),
                                    stop=(ct == KD - 1),
                                )
                            nc.vector.tensor_add(
                                y_sb[:, rt, jc * 512:(jc + 1) * 512],
                                ps[:],
                                bp_bc[:, jc * 512:(jc + 1) * 512],
                            )

                    out_dst = out_d[s * SLAB:(s + 1) * SLAB, :] \
                        .rearrange("(rt p) c -> p rt c", p=P)
                    nc.sync.dma_start(out_dst, y_sb[:])

    nc.compile()
    return nc


def _prep_inputs(x, Wq, bq, Wk, bk, Wv, bv, Wp, bp):
    """Host-side shard prep. Returns in_maps list for 8 cores."""
    bf16 = ml_dtypes.bfloat16
    xf = np.ascontiguousarray(np.asarray(x, dtype=np.float32).reshape(-1, DIM))
    scale = np.float32(1.0 / np.sqrt(64.0))

    wq_b = np.ascontiguousarray((np.asarray(Wq) * scale).astype(bf16))
    wk_b = np.ascontiguousarray(np.asarray(Wk).astype(bf16))
    wv_b = np.ascontiguousarray(np.asarray(Wv).astype(bf16))
    wp_b = np.ascontiguousarray(np.asarray(Wp).astype(bf16))

    bqc = np.ascontiguousarray(np.broadcast_to(
        (np.asarray(bq) * scale).astype(np.float32), (P, DIM)))
    bkc = np.ascontiguousarray(np.broadcast_to(
        np.asarray(bk, dtype=np.float32), (P, DIM)))
    bpc = np.ascontiguousarray(np.broadcast_to(
        np.asarray(bp, dtype=np.float32), (P, DIM)))
    bvc = np.ascontiguousarray(
        np.asarray(bv, dtype=np.float32).reshape(KD, P).T)
    ident = np.eye(64, dtype=bf16)

    shared = {
        "wq": wq_b, "wk": wk_b, "wv": wv_b, "wp": wp_b,
        "bqc": bqc, "bkc": bkc, "bpc": bpc, "bvc": bvc,
        "ident64": ident,
    }
    in_maps = []
    for c in range(N_CORES):
        xs = xf[c * ROWS_PER_CORE:(c + 1) * ROWS_PER_CORE]  # [2048, 1024]
        xt = np.ascontiguousarray(xs.T.astype(bf16))        # [1024, 2048]
        in_maps.append({"xt": xt, **shared})
    return in_maps


def kernel(x, Wq, bq, Wk, bk, Wv, bv, Wp, bp):
    if "nc" not in _CACHE:
        _CACHE["nc"] = _build_graph()
    nc = _CACHE["nc"]

    in_maps = _prep_inputs(x, Wq, bq, Wk, bk, Wv, bv, Wp, bp)
    trace = bool(int(os.environ.get("ATHENA_TRACE", "0")))
    res = run_bass_kernel_spmd(nc, in_maps, core_ids=list(range(N_CORES)),
                               trace=trace)
    _CACHE["last_exec_time_ns"] = res.exec_time_ns

    out = np.concatenate([res.results[c]["out"] for c in range(N_CORES)], axis=0)
    return np.ascontiguousarray(out.reshape(np.asarray(x).shape)
                                .astype(np.float32))


# revision 12
# speedup vs baseline: 1.3493x; 1.0178x over previous
"""Trainium2 Bass kernel for nn_Attention_1580547974448.

Math insight: the reference uses raw .reshape (not a head-split transpose) on
[B,T,H*HD] -> [B,H,T,HD].  With B=4, T=4096, DIM=1024, H=16, HD=64 this makes
each "head" a contiguous 256-row slab of the flattened [B*T, DIM] = [16384,1024]
input: for slab s (rows 256s..256s+255),
    Q = (x_s @ Wq + bq)            viewed row-major as [4096, 64]
    S = Q^T K / sqrt(64)           [64, 64]
    P = softmax(S, axis=-1)
    O = P @ V^T                    [64, 4096], row-major == [256, 1024]
    y_s = O_v @ Wp + bp
i.e. the whole computation is block-diagonal over 64 independent slabs.
We shard 8 slabs (2048 rows) per NeuronCore -> pure data parallel, no
collectives.  Compute dtype bf16 (fp32 PSUM accumulation).

Per-core dataflow (all layouts [partition, free]):
  xt       [128, 8kd, 2048]   x^T, bf16 (host pre-transposed)
  per slab-pair (VT batched at N=512 so LDWEIGHTS hides under the stream):
    vt     [128, 8jt, 512]    V^T for 2 slabs = Wv^T @ xt_pair
  per slab:
    q_nat  [128, 2rt, 1024]   Q rows-on-partitions = xt_slab^T @ Wq + bq (DVE)
    k_nat  same for K
    S psum [64, 64]           sum over (rt,t2) of Q[:,64t2:+64]^T K[:,64t2:+64]
    softmax on free dim (DVE/ACT); WT = W^T via PE transpose
    vvt    [64, 16t2, 256t1]  Vv^T, filled by 16 SBUF->SBUF DMAs (partition
                              crossing move; keeps DVE free)
    O^T    4 chunks per ct into one PSUM [128, 4t3, 64d], single CAST evac
    ovt    [128, 8ct, 256]    Ov^T (col r = 4d + t3)
    y      [128, 2rt, 1024]   = ovt^T @ Wp + bp -> DMA out
Engine split: PE matmuls; DVE Q/K/Y bias-evacs + OT evacs + softmax; ACT VT
bias-evacs + exp; DMA vvt transform.
"""

import os
import sys

import numpy as np
import ml_dtypes

import concourse.bass as bass
import concourse.mybir as mybir
import concourse.tile as tile
from concourse import bacc
from concourse.bass_utils import run_bass_kernel_spmd


def _install_ntff_hook_shim():
    """concourse's trace path does `from antenv.axon_hooks import
    get_axon_ntff_profile_hook`; this container's antenv lacks that
    module.  Provide it: a ctypes hook on the axon PJRT .so when
    available (mirrors trn_agent_boot), else a None hook (concourse
    then skips tracing gracefully)."""
    try:
        import antenv.axon_hooks  # noqa: F401
        return
    except ImportError:
        pass
    import contextlib
    import ctypes
    import types

    state = {"hook": None}

    def _build_hook():
        so_path = "/opt/axon/libaxon_pjrt.so"
        if not os.path.exists(so_path):
            return None
        lib = ctypes.CDLL(so_path)
        if not hasattr(lib, "axon_start_nrt_profile"):
            return None
        lib.axon_start_nrt_profile.argtypes = [
            ctypes.POINTER(ctypes.c_int64), ctypes.c_size_t]
        lib.axon_start_nrt_profile.restype = ctypes.c_int64
        lib.axon_stop_nrt_profile.argtypes = [ctypes.c_char_p]
        lib.axon_stop_nrt_profile.restype = ctypes.c_int64

        @contextlib.contextmanager
        def _hook(output_dir, device_ids):
            import jax
            jax.devices()
            if device_ids:
                ids = (ctypes.c_int64 * len(device_ids))(*device_ids)
                rc = lib.axon_start_nrt_profile(ids, len(device_ids))
            else:
                rc = lib.axon_start_nrt_profile(None, 0)
            if rc != 0:
                raise RuntimeError(f"axon_start_nrt_profile rc={rc}")
            try:
                yield
            finally:
                n = lib.axon_stop_nrt_profile(str(output_dir).encode())
                if n < 0:
                    raise RuntimeError(f"axon_stop_nrt_profile rc={n}")
                print(f"profile: {n} file(s) written to {output_dir}")

        return _hook

    def get_axon_ntff_profile_hook():
        if state["hook"] is None:
            try:
                state["hook"] = _build_hook()
            except Exception:
                state["hook"] = None
        return state["hook"]

    mod = types.ModuleType("antenv.axon_hooks")
    mod.get_axon_ntff_profile_hook = get_axon_ntff_profile_hook
    mod.set_axon_ntff_profile_hook = lambda h: state.update(hook=h)
    sys.modules["antenv.axon_hooks"] = mod


_install_ntff_hook_shim()

P = 128          # SBUF partitions
DIM = 1024       # model dim
KD = DIM // P    # 8 contraction tiles
ROWS_PER_CORE = 2048
SLABS_PER_CORE = 8
SLAB = 256       # rows per slab
N_CORES = 8
BF16 = mybir.dt.bfloat16
F32 = mybir.dt.float32

_CACHE = {}


def _build_graph():
    nc = bacc.Bacc("TRN2", target_bir_lowering=False, debug=False,
                   num_devices=N_CORES)

    xt_d = nc.dram_tensor("xt", [DIM, ROWS_PER_CORE], BF16, kind="ExternalInput")
    w_d = {
        name: nc.dram_tensor(name, [DIM, DIM], BF16, kind="ExternalInput")
        for name in ("wq", "wk", "wv", "wp")
    }
    bqc_d = nc.dram_tensor("bqc", [P, DIM], F32, kind="ExternalInput")
    bkc_d = nc.dram_tensor("bkc", [P, DIM], F32, kind="ExternalInput")
    bpc_d = nc.dram_tensor("bpc", [P, DIM], F32, kind="ExternalInput")
    bvc_d = nc.dram_tensor("bvc", [P, KD], F32, kind="ExternalInput")
    ident_d = nc.dram_tensor("ident64", [64, 64], BF16, kind="ExternalInput")
    out_d = nc.dram_tensor("out", [ROWS_PER_CORE, DIM], F32, kind="ExternalOutput")

    with tile.TileContext(nc) as tc:
        with (
            tc.tile_pool(name="wpool", bufs=1) as wpool,
            tc.tile_pool(name="xpool", bufs=1) as xpool,
            tc.tile_pool(name="bias", bufs=1) as bias_pool,
            tc.tile_pool(name="qk", bufs=2) as qk_pool,
            tc.tile_pool(name="vt", bufs=2) as vt_pool,
            tc.tile_pool(name="vvt", bufs=2) as vvt_pool,
            tc.tile_pool(name="ovt", bufs=2) as ovt_pool,
            tc.tile_pool(name="ysb", bufs=2) as y_pool,
            tc.tile_pool(name="soft", bufs=2) as soft_pool,
            tc.tile_pool(name="ps_proj", bufs=4, space="PSUM") as ps_proj_pool,
            tc.tile_pool(name="ps_s", bufs=1, space="PSUM") as ps_s_pool,
            tc.tile_pool(name="ps_ot", bufs=2, space="PSUM") as ps_ot_pool,
            tc.tile_pool(name="ps_wt", bufs=1, space="PSUM") as ps_wt_pool,
        ):
            # ---- resident tensors -------------------------------------------
            bq_bc = bias_pool.tile([P, DIM], F32, tag="bqc")
            bk_bc = bias_pool.tile([P, DIM], F32, tag="bkc")
            bp_bc = bias_pool.tile([P, DIM], F32, tag="bpc")
            bv_col = bias_pool.tile([P, KD], F32, tag="bvc")
            ident = bias_pool.tile([64, 64], BF16, tag="ident")
            nc.sync.dma_start(bq_bc[:], bqc_d[:])
            nc.sync.dma_start(bk_bc[:], bkc_d[:])
            nc.sync.dma_start(bp_bc[:], bpc_d[:])
            nc.sync.dma_start(bv_col[:], bvc_d[:])
            nc.sync.dma_start(ident[:], ident_d[:])

            # fine-grained loads so slab-0 matmuls can start early
            xt_sb = xpool.tile([P, KD, ROWS_PER_CORE], BF16, tag="xt")
            xt_src = xt_d[:].rearrange("(kd p) r -> p kd r", p=P)
            w_sb = {}
            for name in ("wq", "wk", "wv", "wp"):
                w_sb[name] = wpool.tile([P, KD, DIM], BF16, tag=f"w_{name}",
                                        name=f"w_{name}")
            w_srcs = {name: w_d[name][:].rearrange("(kd p) c -> p kd c", p=P)
                      for name in w_sb}
            for kd in range(KD):
                nc.sync.dma_start(xt_sb[:, kd, 0:512], xt_src[:, kd, 0:512])
                nc.sync.dma_start(w_sb["wv"][:, kd, :], w_srcs["wv"][:, kd, :])
            for kd in range(KD):
                nc.sync.dma_start(w_sb["wq"][:, kd, :], w_srcs["wq"][:, kd, :])
                nc.sync.dma_start(w_sb["wk"][:, kd, :], w_srcs["wk"][:, kd, :])
            for kd in range(KD):
                nc.sync.dma_start(w_sb["wp"][:, kd, :], w_srcs["wp"][:, kd, :])
                for half in range(1, 4):
                    nc.sync.dma_start(
                        xt_sb[:, kd, half * 512:(half + 1) * 512],
                        xt_src[:, kd, half * 512:(half + 1) * 512])

            # ---- per slab-pair pipeline -------------------------------------
            for pair in range(SLABS_PER_CORE // 2):
                p0 = pair * 2 * SLAB  # xt column base of the pair

                # V^T for both slabs of the pair (N=512 keeps PE at stream rate)
                vt = vt_pool.tile([P, KD, 2 * SLAB], BF16, tag="vt")
                for jt in range(KD):
                    ps = ps_proj_pool.tile([P, 512], F32, tag="ps_proj")
                    for kd in range(KD):
                        nc.tensor.matmul(
                            ps[:],
                            w_sb["wv"][:, kd, jt * P:(jt + 1) * P],
                            xt_sb[:, kd, p0: p0 + 512],
                            start=(kd == 0),
                            stop=(kd == KD - 1),
                        )
                    # bias + cast on ScalarE (per-partition bias)
                    nc.scalar.activation(
                        vt[:, jt, :], ps[:],
                        mybir.ActivationFunctionType.Identity,
                        bias=bv_col[:, jt: jt + 1])

                for half in range(2):
                    s = pair * 2 + half
                    c0 = s * SLAB

                    # VvT[e, t1, t2] = VT[64*t2+e, t1] (flat free = 16*t1+t2 = t
                    # so matmul lhsT slices collapse to one free dim).  All
                    # even t2 share src partition base 0, odd share base 64 ->
                    # the 16 per-t2 moves fuse into 2 multi-dim AP copies
                    # (iteration dims [64e, 8j, 256t1] on both sides).
                    vvt = vvt_pool.tile([64, SLAB, 16], BF16, tag="vvt")
                    vvt_j = vvt[:].rearrange("e t1 (j two) -> e two j t1", two=2)
                    src_lo = vt[0:64, :, half * SLAB:(half + 1) * SLAB]
                    src_hi = vt[64:128, :, half * SLAB:(half + 1) * SLAB]
                    nc.scalar.copy(vvt_j[:, 0], src_lo)
                    nc.vector.tensor_copy(vvt_j[:, 1], src_hi)

                    # Q, K natural layout (rows on partitions)
                    q_nat = qk_pool.tile([P, 2, DIM], BF16, tag="q_nat")
                    k_nat = qk_pool.tile([P, 2, DIM], BF16, tag="k_nat")
                    for dst_t, wname, bias_bc in (
                        (q_nat, "wq", bq_bc),
                        (k_nat, "wk", bk_bc),
                    ):
                        for rt in range(2):
                            for jc in range(2):
                                ps = ps_proj_pool.tile([P, 512], F32,
                                                       tag="ps_proj")
                                for kd in range(KD):
                                    nc.tensor.matmul(
                                        ps[:],
                                        xt_sb[:, kd,
                                              c0 + rt * P: c0 + (rt + 1) * P],
                                        w_sb[wname][:, kd,
                                                    jc * 512:(jc + 1) * 512],
                                        start=(kd == 0),
                                        stop=(kd == KD - 1),
                                    )
                                nc.vector.tensor_add(
                                    dst_t[:, rt, jc * 512:(jc + 1) * 512],
                                    ps[:],
                                    bias_bc[:, jc * 512:(jc + 1) * 512],
                                )

                    # S = sum over (rt, t2) of Q_blk^T @ K_blk -> PSUM [64, 64]
                    ps_s = ps_s_pool.tile([64, 64], F32, tag="ps_s")
                    n_acc = 0
                    for rt in range(2):
                        for t2 in range(16):
                            nc.tensor.matmul(
                                ps_s[:],
                                q_nat[:, rt, t2 * 64:(t2 + 1) * 64],
                                k_nat[:, rt, t2 * 64:(t2 + 1) * 64],
                                start=(n_acc == 0),
                                stop=(n_acc == 31),
                            )
                            n_acc += 1

                    # softmax over the free dim (DVE/ACT, overlaps PE)
                    negmax = soft_pool.tile([64, 1], F32, tag="negmax")
                    nc.vector.reduce_max(negmax[:], ps_s[:],
                                         axis=mybir.AxisListType.X, negate=True)
                    p_sb = soft_pool.tile([64, 64], F32, tag="p_sb")
                    rsum = soft_pool.tile([64, 1], F32, tag="rsum")
                    nc.scalar.activation(p_sb[:], ps_s[:],
                                         mybir.ActivationFunctionType.Exp,
                                         bias=negmax[:], accum_out=rsum[:])
                    rinv = soft_pool.tile([64, 1], F32, tag="rinv")
                    nc.vector.reciprocal(rinv[:], rsum[:])
                    w_soft = soft_pool.tile([64, 64], BF16, tag="w_soft")
                    nc.vector.tensor_scalar_mul(w_soft[:], p_sb[:], rinv[:])

                    # WT = W^T via PE transpose
                    ps_wt = ps_wt_pool.tile([64, 64], BF16, tag="ps_wt")
                    nc.tensor.transpose(ps_wt[:], w_soft[:], ident[:])
                    wt_sb = soft_pool.tile([64, 64], BF16, tag="wt_sb")
                    nc.vector.tensor_copy(wt_sb[:], ps_wt[:])

                    # O^T chunks; 4 chunks (t3=0..3) of one ct share a PSUM
                    # tile [128, 4, 64], single CAST evac interleaves into ovt.
                    ovt = ovt_pool.tile([P, KD, SLAB], BF16, tag="ovt")
                    for ct in range(KD):
                        pso = ps_ot_pool.tile([P, 4, 64], F32, tag="ps_ot")
                        for t3 in range(4):
                            c = 8 * t3 + ct
                            # chunk c: t in [128c, 128c+128) -> t1 in [8c,8c+8)
                            # contiguous [64, 8, 16] -> opts to [64, 128]
                            lhs = vvt[:, c * 8:(c + 1) * 8, :]
                            nc.tensor.matmul(
                                pso[:, t3, :],
                                lhs,
                                wt_sb[:],
                                start=True, stop=True,
                            )
                        nc.vector.tensor_copy(
                            ovt[:, ct, :].rearrange("p (d four) -> p d four",
                                                    four=4),
                            pso[:].rearrange("p t3 d -> p d t3"),
                        )

                    # Y = OvT^T @ Wp + bp (natural rows) -> DMA out
                    y_sb = y_pool.tile([P, 2, DIM], F32, tag="y_sb")
                    for rt in range(2):
                        for jc in range(2):
                            ps = ps_proj_pool.tile([P, 512], F32, tag="ps_proj")
                            for ct in range(KD):
                                nc.tensor.matmul(
                                    ps[:],
                                    ovt[:, ct, rt * P:(rt + 1) * P],
                                    w_sb["wp"][:, ct, jc * 512:(jc + 1) * 512],
                                    start=(ct == 0),
                                    stop=(ct == KD - 1),
                                )
                            nc.vector.tensor_add(
                                y_sb[:, rt, jc * 512:(jc + 1) * 512],
                                ps[:],
                                bp_bc[:, jc * 512:(jc + 1) * 512],
                            )

                    out_dst = out_d[s * SLAB:(s + 1) * SLAB, :] \
                        .rearrange("(rt p) c -> p rt c", p=P)
                    nc.sync.dma_start(out_dst, y_sb[:])

    nc.compile()
    return nc


def _prep_inputs(x, Wq, bq, Wk, bk, Wv, bv, Wp, bp):
    """Host-side shard prep. Returns in_maps list for 8 cores."""
    bf16 = ml_dtypes.bfloat16
    xf = np.ascontiguousarray(np.asarray(x, dtype=np.float32).reshape(-1, DIM))
    scale = np.float32(1.0 / np.sqrt(64.0))

    wq_b = np.ascontiguousarray((np.asarray(Wq) * scale).astype(bf16))
    wk_b = np.ascontiguousarray(np.asarray(Wk).astype(bf16))
    wv_b = np.ascontiguousarray(np.asarray(Wv).astype(bf16))
    wp_b = np.ascontiguousarray(np.asarray(Wp).astype(bf16))

    bqc = np.ascontiguousarray(np.broadcast_to(
        (np.asarray(bq) * scale).astype(np.float32), (P, DIM)))
    bkc = np.ascontiguousarray(np.broadcast_to(
        np.asarray(bk, dtype=np.float32), (P, DIM)))
    bpc = np.ascontiguousarray(np.broadcast_to(
        np.asarray(bp, dtype=np.float32), (P, DIM)))
    bvc = np.ascontiguousarray(
        np.asarray(bv, dtype=np.float32).reshape(KD, P).T)
    ident = np.eye(64, dtype=bf16)

    shared = {
        "wq": wq_b, "wk": wk_b, "wv": wv_b, "wp": wp_b,
        "bqc": bqc, "bkc": bkc, "bpc": bpc, "bvc": bvc,
        "ident64": ident,
    }
    in_maps = []
    for c in range(N_CORES):
        xs = xf[c * ROWS_PER_CORE:(c + 1) * ROWS_PER_CORE]  # [2048, 1024]
        xt = np.ascontiguousarray(xs.T.astype(bf16))        # [1024, 2048]
        in_maps.append({"xt": xt, **shared})
    return in_maps


def kernel(x, Wq, bq, Wk, bk, Wv, bv, Wp, bp):
    if "nc" not in _CACHE:
        _CACHE["nc"] = _build_graph()
    nc = _CACHE["nc"]

    in_maps = _prep_inputs(x, Wq, bq, Wk, bk, Wv, bv, Wp, bp)
    trace = bool(int(os.environ.get("ATHENA_TRACE", "0")))
    res = run_bass_kernel_spmd(nc, in_maps, core_ids=list(range(N_CORES)),
                               trace=trace)
    _CACHE["last_exec_time_ns"] = res.exec_time_ns

    out = np.concatenate([res.results[c]["out"] for c in range(N_CORES)], axis=0)
    return np.ascontiguousarray(out.reshape(np.asarray(x).shape)
                                .astype(np.float32))
